# revision 2
# baseline (speedup 1.0000x reference)
"""ChebConv-with-spatial-attention Trainium2 kernel.

out[t,b,m,o] = relu( sum_{k,n,f} cheb[k,n,m] * s_a[b,n,m] * X[b,n,f,t] * Theta[k,f,o] )

Shapes: B=16, N=512, F=32, T=24, K=3, O=64.  fp32 in/out at the host
boundary; bf16 on the wire and in the PE (rel-err gate is 2e-2).

Strategy (8 NeuronCores, data-parallel over batch, 2 batches per core):
  stage 0 (DVE):  A_kb[n,m] = cheb_k[n,m] * s_a_b[n,m]            (SBUF, elementwise)
  stage 1 (PE):   Y[(tj,f), (k,m)] = sum_n X[b,n,tg*4+tj,f] * A_kb[n,m]
                  - lhsT = X block [128n, 128(tj,f)], rhs = A [128n, 512m]
                  - accumulate over 4 n-tiles into PSUM [128, 3*512]
  rearrange (DMA, SBUF->SBUF): Z_t[(k,f), m] = Y[(tj,f), (k,m)]
                  - one 4D-AP dma per (b,tg); puts the whole (k,f)=96
                    stage-2 contraction on the partition axis
  stage 2 (PE):   out_t[o, m] = sum_{(k,f)} Theta2[(k,f), o] * Z_t[(k,f), m]
                  - ONE matmul per (b,t): lhsT = Theta2 [96,64] (shared
                    stationary), rhs = Z_t [96, 512] -> 4x fewer PE rows
                    than the per-(k,tj) quadrant scheme
  relu on evac (ACT), DMA out as bf16 [T, BL, O, N]; host casts/transposes.
"""

import sys

sys.path.insert(0, "/opt/trn_rl_repo")

import numpy as np
import ml_dtypes

import concourse.bacc as bacc
import concourse.tile as tile
from concourse import mybir
from concourse.bass_utils import run_bass_kernel_spmd

B, N, F, T, K, O = 16, 512, 32, 24, 3, 64
NC = 8
BL = B // NC          # batches per core = 2
NT = N // 128         # n tiles = 4
TG = T // 4           # t-groups of 4 = 6
FT = F * T            # 768
KF = K * F            # 96

MM_MODE = "bf16"


def _build_program(mode):
    io_dt = mybir.dt.bfloat16
    nc = bacc.Bacc("TRN2", target_bir_lowering=False, debug=False, num_devices=NC)

    # X pre-transposed on host to [BL, N, T, F] so a [128, 128] slice of the
    # free dim covers 4 consecutive t's of all 32 f's.
    X_d = nc.dram_tensor("X", [BL, N, T * F], io_dt, kind="ExternalInput").ap()
    SA_d = nc.dram_tensor("SA", [BL, N, N], io_dt, kind="ExternalInput").ap()
    CH_d = nc.dram_tensor("CH", [K, N, N], io_dt, kind="ExternalInput").ap()
    # Theta flattened on host to [(k,f) = 96, O].
    TH_d = nc.dram_tensor("TH", [KF, O], io_dt, kind="ExternalInput").ap()
    OUT_d = nc.dram_tensor("OUT", [T, BL, O, N], io_dt, kind="ExternalOutput").ap()

    with tile.TileContext(nc) as tc:
        with (
            tc.tile_pool(name="const", bufs=1) as cpool,
            tc.tile_pool(name="ypsum", bufs=2, space="PSUM") as ypool,
            tc.tile_pool(name="opsum", bufs=1, space="PSUM") as opool,
            tc.tile_pool(name="ysb", bufs=2) as ysbpool,
            tc.tile_pool(name="zsb", bufs=2) as zsbpool,
            tc.tile_pool(name="osb", bufs=2) as osbpool,
        ):
            xsb = cpool.tile([128, BL * NT * FT], io_dt, tag="xsb")
            chsb = cpool.tile([128, K * NT * N], io_dt, tag="chsb")
            sasb = cpool.tile([128, BL * NT * N], io_dt, tag="sasb")
            asb = cpool.tile([128, K * BL * NT * N], io_dt, tag="asb")
            thsb = cpool.tile([KF, O], io_dt, tag="thsb")

            def xoff(b, n4):
                return (b * NT + n4) * FT

            def choff(k, n4):
                return (k * NT + n4) * N

            def saoff(b, n4):
                return (b * NT + n4) * N

            def aoff(k, b, n4):
                return ((k * BL + b) * NT + n4) * N

            # ---- input DMAs (n4-major so the pipeline can start early) ----
            # alternate between the two HWDGE rings (SP / ACT); ACT ring is
            # only used here at kernel start when the ACT engine is idle.
            _ring = [nc.sync, nc.scalar]
            _rr = [0]

            def load(dst, src):
                _ring[_rr[0] % 2].dma_start(dst, src)
                _rr[0] += 1

            load(thsb[:, :], TH_d)
            # b=0's working set first: the first stage-1 group needs all four
            # n-tiles of X[0], cheb, and A[.,0,.] before it can finish
            for b in range(BL):
                for n4 in range(NT):
                    load(
                        xsb[:, xoff(b, n4):xoff(b, n4) + FT],
                        X_d[b, n4 * 128:(n4 + 1) * 128, :],
                    )
                    if b == 0:
                        for k in range(K):
                            load(
                                chsb[:, choff(k, n4):choff(k, n4) + N],
                                CH_d[k, n4 * 128:(n4 + 1) * 128, :],
                            )
                    load(
                        sasb[:, saoff(b, n4):saoff(b, n4) + N],
                        SA_d[b, n4 * 128:(n4 + 1) * 128, :],
                    )

            # ---- stage 0: A = cheb * s_a (DVE) ----
            for b in range(BL):
                for n4 in range(NT):
                    for k in range(K):
                        nc.vector.tensor_mul(
                            asb[:, aoff(k, b, n4):aoff(k, b, n4) + N],
                            chsb[:, choff(k, n4):choff(k, n4) + N],
                            sasb[:, saoff(b, n4):saoff(b, n4) + N],
                        )

            groups = [(b, tg) for b in range(BL) for tg in range(TG)]

            def stage1(b, tg):
                yp = ypool.tile([128, K * N], mybir.dt.float32, tag="yp")
                for n4 in range(NT):
                    xw = xsb[:, xoff(b, n4) + tg * 128: xoff(b, n4) + (tg + 1) * 128]
                    for k in range(K):
                        nc.tensor.matmul(
                            yp[:, k * N:(k + 1) * N],
                            xw,
                            asb[:, aoff(k, b, n4):aoff(k, b, n4) + N],
                            start=(n4 == 0),
                            stop=(n4 == NT - 1),
                        )
                # evacuate PSUM -> SBUF (bf16), split across DVE and ACT
                ysb = ysbpool.tile([128, K * N], io_dt, tag="ysb")
                nc.vector.tensor_copy(ysb[:, 0:1024], yp[:, 0:1024])
                nc.scalar.copy(ysb[:, 1024:1536], yp[:, 1024:1536])
                # rearrange Y[(tj,f), (k,m)] -> Z[(k,f), (tj,m)] so the whole
                # (k,f) contraction lands on partitions (SBUF->SBUF DMA is the
                # only partition-crossing path)
                zsb = zsbpool.tile([KF, 4 * N], io_dt, tag="zsb")
                nc.sync.dma_start(
                    zsb.rearrange("(k f) (tj m) -> k tj f m", f=F, m=N),
                    ysb.rearrange("(tj f) (k m) -> k tj f m", f=F, m=N),
                )
                return zsb

            def stage2(b, tg, zsb):
                op = opool.tile([128, 1024], mybir.dt.float32, tag="op")
                # t = tg*4 + 2*c + pj lives in quadrant (64*pj partitions,
                # 512*c cols); one full-depth (k,f)=96 matmul per t
                for tj in range(4):
                    pj, c = tj % 2, tj // 2
                    nc.tensor.matmul(
                        op[64 * pj:64 * pj + 64, 512 * c:512 * c + 512],
                        thsb[:, :],
                        zsb[:, tj * N:(tj + 1) * N],
                        start=True,
                        stop=True,
                    )
                ob = osbpool.tile([128, 1024], io_dt, tag="ob")
                nc.scalar.activation(ob[:], op[:], mybir.ActivationFunctionType.Relu)
                # SBUF [128,1024] quadrants -> DRAM [4t, O, N] in one DMA on
                # the ACT ring (the relu just ran there, so no extra stall)
                nc.scalar.dma_start(
                    OUT_d[tg * 4:(tg + 1) * 4, b].rearrange("(c pj) o m -> c pj o m", pj=2),
                    ob.rearrange("(pj o) (c m) -> c pj o m", o=O, m=N),
                )

            # software-pipeline: stage2(g-1) is emitted after stage1(g) so the
            # PE never waits on the evac+rearrange of the current group
            prev = None
            for g, (b, tg) in enumerate(groups):
                zsb = stage1(b, tg)
                if prev is not None:
                    stage2(*prev)
                prev = (b, tg, zsb)
            stage2(*prev)

    nc.compile()
    return nc


_prog_cache = {}


def _get_program(mode):
    if mode not in _prog_cache:
        _prog_cache[mode] = _build_program(mode)
    return _prog_cache[mode]


def _prep_in_maps(X, s_a, cheb, Theta):
    np_dt = ml_dtypes.bfloat16
    Xh = np.ascontiguousarray(X.transpose(0, 1, 3, 2)).reshape(B, N, T * F).astype(np_dt)
    sah = np.ascontiguousarray(s_a).astype(np_dt)
    chh = np.ascontiguousarray(cheb).astype(np_dt)
    thh = np.ascontiguousarray(Theta).reshape(KF, O).astype(np_dt)   # [(k,f), O]
    in_maps = []
    for c in range(NC):
        lo, hi = c * BL, (c + 1) * BL
        in_maps.append({
            "X": Xh[lo:hi],
            "SA": sah[lo:hi],
            "CH": chh,
            "TH": thh,
        })
    return in_maps


def kernel(X, s_a, cheb, Theta):
    in_maps = _prep_in_maps(X, s_a, cheb, Theta)
    nc = _get_program(MM_MODE)
    res = run_bass_kernel_spmd(nc, in_maps, list(range(NC)))
    # per-core OUT: bf16 [T, BL, O, N] -> full fp32 [T, B, N, O]
    out = np.concatenate([r["OUT"].astype(np.float32) for r in res.results], axis=1)
    return np.ascontiguousarray(out.transpose(0, 1, 3, 2))
